# revision 2
# baseline (speedup 1.0000x reference)
"""Trainium2 Bass kernel v3: baseline pipeline + batched per-step DMAs.

Identical math/pipeline to the staged baseline (NCHUNK batch chunks, 7-unit
staggered software pipeline, fp16 matmuls, fp32 state + fp16 shadow), with the
DMA traffic restructured:
  * ONE fp16 noise-in DMA per step into a parity-buffered flat tile
    nzflat[64, 2, 512] (was 3 fp32 DMAs into per-chunk tiles)
  * ONE fp32 state-out DMA per step from a parity-buffered flat tile
    yflat[64, 2, 512] that the per-chunk y-updates write slices of
    (was 3 DMAs)
This cuts Sync-engine occupancy ~4x and removes DMA-completion latency from
the per-chunk v-prep path (noise is prefetched one step ahead).
"""

import os
import numpy as np

B, D, W, T = 4096, 64, 256, 256
NCORES = 8
BL = B // NCORES
SIGMA = 0.1

NCHUNK = 3


def _build(dts, zero_bias, nchunk=None, steps=T, bl=BL):
    import concourse.bass as bass  # noqa: F401
    import concourse.mybir as mybir
    import concourse.tile as tile
    from concourse import bacc

    if nchunk is None:
        nchunk = NCHUNK
    f32 = mybir.dt.float32
    f16 = mybir.dt.float16
    Tanh = mybir.ActivationFunctionType.Tanh
    MULT = mybir.AluOpType.mult
    ADD = mybir.AluOpType.add

    dts = np.asarray(dts, np.float32)

    base = bl // nchunk
    rem = bl - base * nchunk
    csizes = [base + (1 if c < rem else 0) for c in range(nchunk)]
    los = [sum(csizes[:c]) for c in range(nchunk)]
    chunks = list(range(nchunk))

    nc = bacc.Bacc("TRN2", target_bir_lowering=False, debug=False)

    y0_d = nc.dram_tensor("y0t", [D, bl], f32, kind="ExternalInput")
    y0h_d = nc.dram_tensor("y0th", [D, bl], f16, kind="ExternalInput")
    nz_d = nc.dram_tensor("nz", [steps, D, bl], f16, kind="ExternalInput")
    win_d = nc.dram_tensor("wint", [D, W], f16, kind="ExternalInput")
    wh_d = nc.dram_tensor("wht", [2, 2, 128, W], f16, kind="ExternalInput")
    wout_d = nc.dram_tensor("woutt", [2, 128, D], f16, kind="ExternalInput")
    if not zero_bias:
        bias_d = nc.dram_tensor("biases", [3, 128, 2], f32, kind="ExternalInput")
    out_d = nc.dram_tensor("outt", [steps, D, bl], f32, kind="ExternalOutput")

    mm = nc.tensor.matmul

    with tile.TileContext(nc) as tc:
        with (
            tc.tile_pool(name="const", bufs=1) as const,
            tc.tile_pool(name="hbuf", bufs=3) as hbuf,
            tc.tile_pool(name="state", bufs=4) as st,
            tc.tile_pool(name="psum", bufs=1, space="PSUM") as ps,
        ):
            # ---- constants ----
            win_s = const.tile([D, W], f16)
            nc.sync.dma_start(out=win_s[:], in_=win_d[:])
            wh_s = const.tile([128, 2, 2, W], f16)
            for li in range(2):
                for k in range(2):
                    nc.sync.dma_start(out=wh_s[:, li, k, :], in_=wh_d[li, k])
            wout_s = const.tile([128, 2, D], f16)
            for k in range(2):
                nc.sync.dma_start(out=wout_s[:, k, :], in_=wout_d[k])
            if not zero_bias:
                bias_s = const.tile([128, 3, 2], f32)
                for j in range(3):
                    nc.sync.dma_start(out=bias_s[:, j, :], in_=bias_d[j])

            # ---- persistent parity-buffered flat tiles ----
            yflat = st.tile([D, 2, bl], f32, tag="yflat")
            nzflat = st.tile([D, 2, bl], f16, tag="nzflat")
            nc.sync.dma_start(out=yflat[:, 0, :], in_=y0_d[:])
            nc.sync.dma_start(out=nzflat[:, 0, :], in_=nz_d[0])

            # fp16 state shadow (feeds matmuls); initial from host
            yhcur = []
            for c in chunks:
                csz, lo = csizes[c], los[c]
                yh_t = st.tile([D, csz], f16, tag=f"yh{c}")
                nc.sync.dma_start(out=yh_t[:], in_=y0h_d[:, lo:lo + csz])
                yhcur.append(yh_t)

            def tanh_layer(h_sb, h_ps, li):
                if zero_bias:
                    nc.scalar.activation(
                        out=h_sb.rearrange("p a b -> p (a b)"),
                        in_=h_ps.rearrange("p a b -> p (a b)"),
                        func=Tanh)
                else:
                    for m in range(2):
                        nc.scalar.activation(
                            out=h_sb[:, m, :], in_=h_ps[:, m, :], func=Tanh,
                            bias=bias_s[:, li, m:m + 1])

            NU = 7
            # measured best stagger: tanh units of the 3 chunks interleave
            # evenly with minimal exposed recurrence stalls
            LAGS = (0, 1, 2) if nchunk == 3 else tuple(
                c * NU // nchunk for c in chunks)
            live = {c: {} for c in chunks}

            def unit(c, t, u):
                if u >= 7:
                    return  # padding slots (NU > 7): emission-schedule spacing
                dt = float(dts[t])
                csz, lo = csizes[c], los[c]
                lv = live[c]
                if u == 0:
                    if c == 0 and t + 1 < steps:
                        nc.sync.dma_start(out=nzflat[:, (t + 1) % 2, :],
                                          in_=nz_d[t + 1])
                    # own tag: keeps the L1 matmuls' only late wait the yh RAW
                    # (WAR is vs T1(t) long ago) so their LDWEIGHTS preloads
                    lv['h1p'] = ps.tile([128, 2, csz], f32, tag=f"hCp{c}",
                                        name=f"h1p_{c}")
                    mm(lv['h1p'][:, 0, :], win_s[:, 0:128], yhcur[c][:],
                       start=True, stop=True)
                    mm(lv['h1p'][:, 1, :], win_s[:, 128:256], yhcur[c][:],
                       start=True, stop=True)
                    # v = (1-dt)*y + noise, early: off the serial chain
                    lv['v'] = st.tile([D, csz], f32, tag=f"v{c}", name=f"v_{c}")
                    nc.vector.scalar_tensor_tensor(
                        out=lv['v'][:], in0=yflat[:, t % 2, lo:lo + csz],
                        scalar=1.0 - dt, in1=nzflat[:, t % 2, lo:lo + csz],
                        op0=MULT, op1=ADD)
                elif u == 1:
                    hs = hbuf.tile([128, 2, csz], f16, tag=f"h1{c}",
                                   name=f"h1_{c}")
                    tanh_layer(hs, lv['h1p'], 0)
                    lv['h1'] = hs
                elif u in (3, 5):
                    li = (u - 1) // 2
                    hs = hbuf.tile([128, 2, csz], f16, tag=f"h{li + 1}{c}",
                                   name=f"h{li + 1}_{c}")
                    tanh_layer(hs, lv[f'h{li + 1}p'], li)
                    lv[f'h{li + 1}'] = hs
                elif u in (2, 4):
                    li = (u - 2) // 2
                    hp = ps.tile([128, 2, csz], f32, tag=f"hAp{c}",
                                 name=f"h{li + 2}p_{c}")
                    hprev = lv[f'h{li + 1}']
                    for m in range(2):
                        for k in range(2):
                            mm(hp[:, m, :],
                               wh_s[:, li, k, m * 128:(m + 1) * 128],
                               hprev[:, k, :], start=(k == 0), stop=(k == 1))
                    lv[f'h{li + 2}p'] = hp
                elif u == 6:
                    ypt = ps.tile([D, csz], f32, tag=f"hAp{c}", name=f"yp_{c}")
                    mm(ypt[:], wout_s[:, 0, :], lv['h3'][:, 0, :],
                       start=True, stop=False)
                    mm(ypt[:], wout_s[:, 1, :], lv['h3'][:, 1, :],
                       start=False, stop=True)
                    # fp16 shadow first (feeds next step's matmuls ASAP)
                    yh_nx = st.tile([D, csz], f16, tag=f"yh{c}",
                                    name=f"yh_{c}")
                    nc.vector.scalar_tensor_tensor(
                        out=yh_nx[:], in0=ypt[:], scalar=dt, in1=lv['v'][:],
                        op0=MULT, op1=ADD)
                    yhcur[c] = yh_nx
                    nc.vector.scalar_tensor_tensor(
                        out=yflat[:, (t + 1) % 2, lo:lo + csz],
                        in0=ypt[:], scalar=dt, in1=lv['v'][:],
                        op0=MULT, op1=ADD)
                    if c == nchunk - 1:
                        nc.sync.dma_start(out=out_d[t],
                                          in_=yflat[:, (t + 1) % 2, :])

            total = steps * NU + max(LAGS)
            for g in range(total):
                for c in chunks:
                    gg = g - LAGS[c]
                    if 0 <= gg < steps * NU:
                        t, u = divmod(gg, NU)
                        unit(c, t, u)
    nc.compile()
    return nc


def _host_prep(ts, y0, dW, w_in, b_in, w_h, b_h, w_out, b_out):
    f = np.float32
    h = np.float16
    ts = np.asarray(ts, f)
    dts = (ts[1:] - ts[:-1]).astype(f)
    assert dts.shape[0] == T

    zero_bias = (not np.any(b_in)) and (not np.any(b_h))

    scale = (SIGMA * np.sqrt(dts)).astype(f)
    drift = (dts[:, None] * np.asarray(b_out, f)[None, :]).astype(f)

    w_inT = np.ascontiguousarray(np.asarray(w_in, f).T.astype(h))
    whT = np.ascontiguousarray(
        np.stack([np.asarray(w_h[i], f).T.reshape(2, 128, W) for i in range(2)])
    ).astype(h)
    w_outT = np.ascontiguousarray(np.asarray(w_out, f).T.reshape(2, 128, D)).astype(h)

    biases = np.zeros((3, 128, 2), f)
    biases[0] = np.asarray(b_in, f).reshape(2, 128).T
    biases[1] = np.asarray(b_h[0], f).reshape(2, 128).T
    biases[2] = np.asarray(b_h[1], f).reshape(2, 128).T

    y0 = np.asarray(y0, f)
    dW = np.asarray(dW, f)

    in_maps = []
    for c in range(NCORES):
        lo = c * BL
        nzc = dW[:, lo:lo + BL, :] * scale[:, None, None] + drift[:, None, :]
        nzc = np.ascontiguousarray(nzc.transpose(0, 2, 1)).astype(h)
        y0tc = np.ascontiguousarray(y0[lo:lo + BL].T)
        m = {
            "y0t": y0tc,
            "y0th": y0tc.astype(h),
            "nz": nzc,
            "wint": w_inT,
            "wht": whT,
            "woutt": w_outT,
        }
        if not zero_bias:
            m["biases"] = biases
        in_maps.append(m)
    return in_maps, dts, zero_bias


_NC_CACHE = {}
TRACE = False
LAST_RESULT = None


def kernel(ts, y0, dW, w_in, b_in, w_h, b_h, w_out, b_out):
    global LAST_RESULT
    from concourse.bass_utils import run_bass_kernel_spmd

    in_maps, dts, zero_bias = _host_prep(
        ts, y0, dW, w_in, b_in, w_h, b_h, w_out, b_out)

    key = (zero_bias, NCHUNK, np.asarray(dts).tobytes())
    nc = _NC_CACHE.get(key)
    if nc is None:
        nc = _build(dts, zero_bias)
        _NC_CACHE[key] = nc

    res = run_bass_kernel_spmd(nc, in_maps, core_ids=list(range(NCORES)),
                               trace=TRACE)
    LAST_RESULT = res

    out = np.empty((T + 1, B, D), np.float32)
    out[0] = np.asarray(y0, np.float32)
    for c in range(NCORES):
        lo = c * BL
        out[1:, lo:lo + BL, :] = res.results[c]["outt"].transpose(0, 2, 1)
    return out
